# revision 7
# baseline (speedup 1.0000x reference)
"""Trainium2 Bass kernel for CustomSAGEConv (GNN mean-aggregation message passing).

  out = normalize( mean_agg(x[row] -> col) @ W_agg.T + x @ W_lin.T )

Strategy (8 NeuronCores, SPMD single program):
  - Host: partition the 100K nodes into 784 blocks of 128 (8 cores x 98
    blocks), balancing per-(block, source-range-group) in-edge counts so every
    block fits a uniform chunk grid of (6,6,6,1) 128-edge chunks for the 4
    source ranges (dma_gather uses int16 indices -> x is split into 4 tables
    of <=32768 rows). Edges are routed to the core/block owning their dest
    (col); within a block they are grouped by source range; padding slots get
    loc=255 (ignored by the one-hot) and index 0.
  - Device, per super-block of B=7 blocks:
      1. 4 dma_gather instructions (one per range table) fetch all B*Mg[g]*128
         source rows; gathered row j lands at msg[j%128, j//128, :].
      2. one-hot S[e, c, :] = (loc == c) built once per super-block on DVE.
      3. per block: 19 matmuls accumulate PSUM[c, :] += S_c.T @ msg_c.
      4. agg = summed * invdeg; PE-transpose; out = agg@W_agg.T + x@W_lin.T;
         row L2-normalize; DMA out.
  - Host: inverse-permute rows back to original node order.
"""

import sys

sys.path.insert(0, "/opt/trn_rl_repo")

import numpy as np

P = 128
TBL = 32768
MG = (6, 6, 6, 1)          # chunks per block for each source-range group
B = 7                      # blocks per super-block
CAPS = tuple(m * P for m in MG)


# ---------------------------------------------------------------- host prep

def _host_prep(x, W_lin, W_agg, edge_index, ncores, bpc, dt_np):
    """Build per-core device inputs."""
    N, D = x.shape
    assert D == P
    NBLK = ncores * bpc
    NPAD = NBLK * P
    assert N <= NPAD
    assert bpc % B == 0
    NSB = bpc // B

    row = np.ascontiguousarray(edge_index[0]).astype(np.int32)
    col = np.ascontiguousarray(edge_index[1]).astype(np.int32)
    E = row.shape[0]
    grp = (row // TBL).astype(np.int32)
    NG = 4
    assert grp.max() < NG

    # per-node per-group in-degree
    indeg4 = np.bincount(col.astype(np.int64) * NG + grp,
                         minlength=NPAD * NG).reshape(NPAD, NG)
    deg = indeg4.sum(1)

    # initial node->block: degree-sorted snake round robin (balances totals;
    # group splits then come out near the multinomial mean)
    order = np.argsort(-deg, kind="stable")
    seq = np.arange(NPAD, dtype=np.int64)
    cyc, pos = seq // NBLK, seq % NBLK
    snake = np.where(cyc % 2 == 0, pos, NBLK - 1 - pos).astype(np.int32)
    blk_of = np.empty(NPAD, np.int32)
    blk_of[order] = snake

    n_bg = np.bincount(blk_of[col].astype(np.int64) * NG + grp,
                       minlength=NBLK * NG).reshape(NBLK, NG)
    caps = np.array(CAPS, np.int64)
    for _ in range(3000):
        over = n_bg - caps[None, :]
        flat = int(np.argmax(over))
        b_hi, g = flat // NG, flat % NG
        if over[b_hi, g] <= 0:
            break
        nodes_hi = np.where(blk_of == b_hi)[0]
        n1 = nodes_hi[np.argmax(indeg4[nodes_hi, g])]
        done = False
        for b_lo in np.argsort(n_bg[:, g])[:32]:
            if b_lo == b_hi:
                continue
            nodes_lo = np.where(blk_of == b_lo)[0]
            n2 = nodes_lo[np.argmin(indeg4[nodes_lo, g])]
            d = indeg4[n1] - indeg4[n2]
            if d[g] <= 0:
                continue
            if np.all(n_bg[b_lo] + d <= caps) and np.all(n_bg[b_hi] - d <= caps):
                blk_of[n1], blk_of[n2] = b_lo, b_hi
                n_bg[b_hi] -= d
                n_bg[b_lo] += d
                done = True
                break
        if not done:
            raise RuntimeError("block/group balance repair failed")
    assert np.all(n_bg <= caps[None, :]), "balance failed"

    # node -> (block, loc); slot->node map
    o2 = np.argsort(blk_of, kind="stable")
    loc_of = np.empty(NPAD, np.int32)
    loc_of[o2] = (np.arange(NPAD, dtype=np.int64) % P).astype(np.int32)
    node_of_slot = o2

    # ---- edge slot placement -------------------------------------------
    # layout per core: [sb][g][b_local][slot within Mg[g]*128]
    eb = blk_of[col]
    el = loc_of[col]
    eo = np.lexsort((row, grp, eb))
    eb_s, el_s, row_s, eg_s = eb[eo], el[eo], row[eo], grp[eo]

    key = eb_s.astype(np.int64) * NG + eg_s
    cnt = np.bincount(key, minlength=NBLK * NG)
    starts = np.concatenate([[0], np.cumsum(cnt)[:-1]])
    within = np.arange(E, dtype=np.int64) - np.repeat(starts, cnt)

    CH_SB = B * sum(MG)                 # chunks per super-block (133)
    SLOTS_SB = CH_SB * P                # 17024
    SLOTS_CORE = NSB * SLOTS_SB         # 238336
    CH_CORE = NSB * CH_SB
    W_CORE = SLOTS_CORE // 16
    cumMgB = np.concatenate([[0], np.cumsum([B * m for m in MG])])  # chunks
    # slot base for (block b, group g), relative to its core
    b_all = np.arange(NBLK, dtype=np.int64)
    b_in_core = b_all % bpc
    sb_of = b_in_core // B
    bl_of = b_in_core % B
    base_bg = np.empty((NBLK, NG), np.int64)
    for g in range(NG):
        base_bg[:, g] = (sb_of * SLOTS_SB + cumMgB[g] * P
                         + bl_of * MG[g] * P)
    slot_core = base_bg.reshape(-1)[key] + within
    core_of_edge = (eb_s // bpc).astype(np.int64)

    idx16 = np.zeros((ncores, SLOTS_CORE), np.int16)
    locs = np.full((ncores, SLOTS_CORE), 255.0, np.float32)
    flat = core_of_edge * SLOTS_CORE + slot_core
    idx16.reshape(-1)[flat] = (row_s - eg_s * TBL).astype(np.int16)
    locs.reshape(-1)[flat] = el_s

    # idx j -> [j%16, j//16], replicated to 128 partitions
    idx_wrap = idx16.reshape(ncores, W_CORE, 16).transpose(0, 2, 1)
    idx_rep = np.ascontiguousarray(np.tile(idx_wrap, (1, 8, 1)))
    # locs chunk-major: slot = c*128 + p -> locs_T[p, c]
    locs_T = np.ascontiguousarray(
        locs.reshape(ncores, CH_CORE, P).transpose(0, 2, 1)).astype(dt_np)

    # gather tables (node-id order, split by range)
    x_pad = np.zeros((NPAD, P), np.float32)
    x_pad[:N] = x
    xg = np.ascontiguousarray(x_pad.astype(dt_np))
    xg_tables = [xg[0:TBL], xg[TBL:2 * TBL], xg[2 * TBL:3 * TBL],
                 xg[3 * TBL:NPAD]]

    # inverse in-degree per (core, loc, block)
    invdeg = (1.0 / np.maximum(deg, 1.0)).astype(np.float32)
    invdeg_slot = invdeg[node_of_slot]
    invdeg_T = np.ascontiguousarray(
        invdeg_slot.reshape(ncores, bpc, P).transpose(0, 2, 1))

    # per-core transposed x in (block, loc) order
    xt_all = x_pad[node_of_slot].astype(dt_np)
    xt_cores = np.ascontiguousarray(
        xt_all.reshape(ncores, bpc * P, P).transpose(0, 2, 1))

    waggT = np.ascontiguousarray(W_agg.T).astype(dt_np)
    wlinT = np.ascontiguousarray(W_lin.T).astype(dt_np)
    iota = np.tile(np.arange(P, dtype=np.float64), (P, 1)).astype(dt_np)
    ident = np.eye(P, dtype=np.float64).astype(dt_np)

    in_maps = []
    for k in range(ncores):
        m = {
            "idxs": idx_rep[k],
            "locs": locs_T[k],
            "invdeg": invdeg_T[k],
            "xt": xt_cores[k],
            "wagg": waggT,
            "wlin": wlinT,
            "iota": iota,
            "ident": ident,
        }
        for g in range(NG):
            m[f"xg{g}"] = xg_tables[g]
        in_maps.append(m)
    return in_maps, node_of_slot


# ---------------------------------------------------------------- device program

def _build_nc(bpc, dt_np, debug=False):
    import concourse.bass as bass  # noqa: F401
    import concourse.bacc as bacc
    import concourse.mybir as mybir
    import concourse.tile as tile

    dt = mybir.dt.from_np(np.dtype(dt_np))
    f32 = mybir.dt.float32
    NB = bpc
    NCN = NB * P
    NSB = bpc // B
    NG = 4
    CH_SB = B * sum(MG)
    SLOTS_SB = CH_SB * P
    CH_CORE = NSB * CH_SB
    W_CORE = NSB * SLOTS_SB // 16
    cumMgB = [0]
    for m in MG:
        cumMgB.append(cumMgB[-1] + B * m)
    tbl_rows = [TBL, TBL, TBL, NCN * 8 - 3 * TBL]

    nc = bacc.Bacc("TRN2", target_bir_lowering=False, debug=debug,
                   num_swdge_queues=4)

    xg_d = [nc.dram_tensor(f"xg{g}", [tbl_rows[g], P], dt,
                           kind="ExternalInput") for g in range(NG)]
    idxs_d = nc.dram_tensor("idxs", [P, W_CORE], mybir.dt.int16,
                            kind="ExternalInput")
    locs_d = nc.dram_tensor("locs", [P, CH_CORE], dt, kind="ExternalInput")
    invdeg_d = nc.dram_tensor("invdeg", [P, NB], f32, kind="ExternalInput")
    xt_d = nc.dram_tensor("xt", [P, NCN], dt, kind="ExternalInput")
    wagg_d = nc.dram_tensor("wagg", [P, P], dt, kind="ExternalInput")
    wlin_d = nc.dram_tensor("wlin", [P, P], dt, kind="ExternalInput")
    iota_d = nc.dram_tensor("iota", [P, P], dt, kind="ExternalInput")
    ident_d = nc.dram_tensor("ident", [P, P], dt, kind="ExternalInput")
    out_d = nc.dram_tensor("out", [NCN, P], f32, kind="ExternalOutput")

    AF = mybir.ActivationFunctionType
    OP = mybir.AluOpType

    with tile.TileContext(nc) as tc:
        with tc.tile_pool(name="const", bufs=1) as cp, \
             tc.tile_pool(name="msg", bufs=2) as mp, \
             tc.tile_pool(name="spool", bufs=2) as spool, \
             tc.tile_pool(name="xpool", bufs=2) as xp, \
             tc.tile_pool(name="blk", bufs=2) as bp, \
             tc.tile_pool(name="psum", bufs=2, space="PSUM") as pp:

            idxs_t = cp.tile([P, W_CORE], mybir.dt.int16)
            nc.sync.dma_start(out=idxs_t[:], in_=idxs_d[:])
            locs_t = cp.tile([P, CH_CORE], dt)
            nc.sync.dma_start(out=locs_t[:], in_=locs_d[:])
            invdeg_t = cp.tile([P, NB], f32)
            nc.sync.dma_start(out=invdeg_t[:], in_=invdeg_d[:])
            iota_t = cp.tile([P, P], dt)
            nc.sync.dma_start(out=iota_t[:], in_=iota_d[:])
            ident_t = cp.tile([P, P], dt)
            nc.sync.dma_start(out=ident_t[:], in_=ident_d[:])
            wagg_t = cp.tile([P, P], dt)
            nc.sync.dma_start(out=wagg_t[:], in_=wagg_d[:])
            wlin_t = cp.tile([P, P], dt)
            nc.sync.dma_start(out=wlin_t[:], in_=wlin_d[:])

            GCH = 5  # chunks per dma_gather (640 idxs = 41 descs/engine:
                     # two gathers fit in a queue's ring -> descgen pipelines
                     # with the previous gather's transfer)
            qq = 0
            for sb in range(NSB):
                # 1. gather the super-block's source rows: sub-gathers of
                # GCH chunks per range table, spread over the 4 SWDGE queues
                # (each queue = its own Q7 cpu pair + SDMA ring)
                msgs = []
                for g in range(NG):
                    nch = B * MG[g]
                    mg_t = mp.tile([P, nch, P], dt, tag=f"m{g}")
                    off = sb * SLOTS_SB + cumMgB[g] * P
                    for c0 in range(0, nch, GCH):
                        n_sub = min(GCH, nch - c0) * P
                        o = off + c0 * P
                        nc.gpsimd.dma_gather(
                            mg_t[:, c0:c0 + n_sub // P, :], xg_d[g][:],
                            idxs_t[:, o // 16:(o + n_sub) // 16],
                            n_sub, n_sub, P, queue_num=qq % 4)
                        qq += 1
                    msgs.append(mg_t)

                # 2. one-hot S[e, c_chunk, :] = (loc == iota)
                S_t = spool.tile([P, CH_SB, P], dt, tag="S")
                c0 = sb * CH_SB
                nc.vector.tensor_tensor(
                    out=S_t[:],
                    in0=locs_t[:, c0:c0 + CH_SB].to_broadcast([P, CH_SB, P]),
                    in1=iota_t[:, None, :].to_broadcast([P, CH_SB, P]),
                    op=OP.is_equal)

                xt_t = xp.tile([P, B * P], dt, tag="xt")
                nc.sync.dma_start(
                    out=xt_t[:], in_=xt_d[:, sb * B * P:(sb + 1) * B * P])

                for bl in range(B):
                    b = sb * B + bl
                    # 3. scatter-accumulate: acc[c, :] += S_c.T @ msg_c
                    acc_p = pp.tile([P, P], f32, tag="acc")
                    mm, total_mm = 0, sum(MG)
                    for g in range(NG):
                        for m in range(MG[g]):
                            c_local = cumMgB[g] + bl * MG[g] + m
                            nc.tensor.matmul(
                                out=acc_p[:], lhsT=S_t[:, c_local, :],
                                rhs=msgs[g][:, bl * MG[g] + m, :],
                                start=(mm == 0), stop=(mm == total_mm - 1))
                            mm += 1

                    # 4. agg = summed * invdeg
                    agg_t = bp.tile([P, P], dt, tag="agg")
                    nc.vector.tensor_scalar(
                        out=agg_t[:], in0=acc_p[:],
                        scalar1=invdeg_t[:, b:b + 1], scalar2=None,
                        op0=OP.mult)

                    # 5. transpose agg; out = agg @ W_agg.T + x @ W_lin.T
                    aggT_p = pp.tile([P, P], dt, tag="aggTp")
                    nc.tensor.transpose(out=aggT_p[:], in_=agg_t[:],
                                        identity=ident_t[:])
                    aggT_t = bp.tile([P, P], dt, tag="aggT")
                    nc.vector.tensor_copy(out=aggT_t[:], in_=aggT_p[:])
                    out_p = pp.tile([P, P], f32, tag="out")
                    nc.tensor.matmul(out=out_p[:], lhsT=aggT_t[:],
                                     rhs=wagg_t[:], start=True, stop=False)
                    nc.tensor.matmul(out=out_p[:],
                                     lhsT=xt_t[:, bl * P:(bl + 1) * P],
                                     rhs=wlin_t[:], start=False, stop=True)

                    # 6. L2 normalize rows
                    sq_t = bp.tile([P, P], f32, tag="sq")
                    ss_t = bp.tile([P, 1], f32, tag="ss")
                    nc.scalar.activation(out=sq_t[:], in_=out_p[:],
                                         func=AF.Square, accum_out=ss_t[:])
                    nrm_t = bp.tile([P, 1], f32, tag="nrm")
                    nc.scalar.sqrt(out=nrm_t[:], in_=ss_t[:])
                    nrmc_t = bp.tile([P, 1], f32, tag="nrmc")
                    nc.vector.tensor_scalar_max(nrmc_t[:], nrm_t[:], 1e-12)
                    inv_t = bp.tile([P, 1], f32, tag="inv")
                    nc.vector.reciprocal(out=inv_t[:], in_=nrmc_t[:])
                    outs_t = bp.tile([P, P], f32, tag="outs")
                    nc.vector.tensor_scalar(
                        out=outs_t[:], in0=out_p[:],
                        scalar1=inv_t[:, :1], scalar2=None, op0=OP.mult)
                    nc.sync.dma_start(out=out_d[b * P:(b + 1) * P, :],
                                      in_=outs_t[:])

    return nc


# ---------------------------------------------------------------- entry point

def _run(x, W_lin, W_agg, edge_index, ncores, bpc, dt_np, trace=False):
    from concourse import bass_utils

    in_maps, node_of_slot = _host_prep(
        x, W_lin, W_agg, edge_index, ncores, bpc, dt_np)
    nc = _build_nc(bpc, dt_np)
    nc.compile()
    res = bass_utils.run_bass_kernel_spmd(
        nc, in_maps, core_ids=list(range(ncores)), trace=trace)
    outs = np.concatenate([r["out"] for r in res.results], axis=0)
    N = x.shape[0]
    out_pad = np.empty((len(node_of_slot), P), np.float32)
    out_pad[node_of_slot] = outs
    return out_pad[:N], res


def kernel(x, W_lin, W_agg, edge_index):
    import os
    x = np.ascontiguousarray(x, dtype=np.float32)
    W_lin = np.ascontiguousarray(W_lin, dtype=np.float32)
    W_agg = np.ascontiguousarray(W_agg, dtype=np.float32)
    dt_env = os.environ.get("KERNEL_DTYPE", "float16")
    if dt_env == "bfloat16":
        import ml_dtypes
        dt_np = ml_dtypes.bfloat16
    elif dt_env == "float16":
        dt_np = np.float16
    else:
        dt_np = np.float32
    trace = os.environ.get("KERNEL_TRACE", "0") == "1"
    if trace:
        try:
            sys.path.insert(0, os.path.dirname(os.path.abspath(__file__)))
            import ntff_shim  # noqa: F401
        except Exception:
            pass
    out, res = _run(x, W_lin, W_agg, edge_index, ncores=8, bpc=98,
                    dt_np=dt_np, trace=trace)
    if res.exec_time_ns is not None:
        print(f"HW exec time: {res.exec_time_ns} ns")
    return out


# revision 8
# speedup vs baseline: 1.0702x; 1.0702x over previous
"""Trainium2 Bass kernel for CustomSAGEConv (GNN mean-aggregation message passing).

  out = normalize( mean_agg(x[row] -> col) @ W_agg.T + x @ W_lin.T )

Strategy (8 NeuronCores, SPMD single program):
  - Host: partition the 100K nodes into 784 blocks of 128 (8 cores x 98
    blocks), balancing per-(block, source-range-group) in-edge counts so every
    block fits a uniform chunk grid of (6,6,6,1) 128-edge chunks for the 4
    source ranges (dma_gather uses int16 indices -> x is split into 4 tables
    of <=32768 rows). Edges are routed to the core/block owning their dest
    (col); within a block they are grouped by source range; padding slots get
    loc=255 (ignored by the one-hot) and index 0.
  - Device, per super-block of B=7 blocks:
      1. 4 dma_gather instructions (one per range table) fetch all B*Mg[g]*128
         source rows; gathered row j lands at msg[j%128, j//128, :].
      2. one-hot S[e, c, :] = (loc == c) built once per super-block on DVE.
      3. per block: 19 matmuls accumulate PSUM[c, :] += S_c.T @ msg_c.
      4. agg = summed * invdeg; PE-transpose; out = agg@W_agg.T + x@W_lin.T;
         row L2-normalize; DMA out.
  - Host: inverse-permute rows back to original node order.
"""

import sys

sys.path.insert(0, "/opt/trn_rl_repo")

import numpy as np

P = 128
TBL = 32768
MG = (6, 6, 6, 1)          # chunks per block for each source-range group
B = 7                      # blocks per super-block
CAPS = tuple(m * P for m in MG)


# ---------------------------------------------------------------- host prep

def _host_prep(x, W_lin, W_agg, edge_index, ncores, bpc, dt_np):
    """Build per-core device inputs."""
    N, D = x.shape
    assert D == P
    NBLK = ncores * bpc
    NPAD = NBLK * P
    assert N <= NPAD
    assert bpc % B == 0
    NSB = bpc // B

    row = np.ascontiguousarray(edge_index[0]).astype(np.int32)
    col = np.ascontiguousarray(edge_index[1]).astype(np.int32)
    E = row.shape[0]
    grp = (row // TBL).astype(np.int32)
    NG = 4
    assert grp.max() < NG

    # per-node per-group in-degree
    indeg4 = np.bincount(col.astype(np.int64) * NG + grp,
                         minlength=NPAD * NG).reshape(NPAD, NG)
    deg = indeg4.sum(1)

    # initial node->block: degree-sorted snake round robin (balances totals;
    # group splits then come out near the multinomial mean)
    order = np.argsort(-deg, kind="stable")
    seq = np.arange(NPAD, dtype=np.int64)
    cyc, pos = seq // NBLK, seq % NBLK
    snake = np.where(cyc % 2 == 0, pos, NBLK - 1 - pos).astype(np.int32)
    blk_of = np.empty(NPAD, np.int32)
    blk_of[order] = snake

    n_bg = np.bincount(blk_of[col].astype(np.int64) * NG + grp,
                       minlength=NBLK * NG).reshape(NBLK, NG)
    caps = np.array(CAPS, np.int64)
    for _ in range(3000):
        over = n_bg - caps[None, :]
        flat = int(np.argmax(over))
        b_hi, g = flat // NG, flat % NG
        if over[b_hi, g] <= 0:
            break
        nodes_hi = np.where(blk_of == b_hi)[0]
        n1 = nodes_hi[np.argmax(indeg4[nodes_hi, g])]
        done = False
        for b_lo in np.argsort(n_bg[:, g])[:32]:
            if b_lo == b_hi:
                continue
            nodes_lo = np.where(blk_of == b_lo)[0]
            n2 = nodes_lo[np.argmin(indeg4[nodes_lo, g])]
            d = indeg4[n1] - indeg4[n2]
            if d[g] <= 0:
                continue
            if np.all(n_bg[b_lo] + d <= caps) and np.all(n_bg[b_hi] - d <= caps):
                blk_of[n1], blk_of[n2] = b_lo, b_hi
                n_bg[b_hi] -= d
                n_bg[b_lo] += d
                done = True
                break
        if not done:
            raise RuntimeError("block/group balance repair failed")
    assert np.all(n_bg <= caps[None, :]), "balance failed"

    # node -> (block, loc); slot->node map
    o2 = np.argsort(blk_of, kind="stable")
    loc_of = np.empty(NPAD, np.int32)
    loc_of[o2] = (np.arange(NPAD, dtype=np.int64) % P).astype(np.int32)
    node_of_slot = o2

    # ---- edge slot placement -------------------------------------------
    # layout per core: [sb][g][b_local][slot within Mg[g]*128]
    eb = blk_of[col]
    el = loc_of[col]
    eo = np.lexsort((row, grp, eb))
    eb_s, el_s, row_s, eg_s = eb[eo], el[eo], row[eo], grp[eo]

    key = eb_s.astype(np.int64) * NG + eg_s
    cnt = np.bincount(key, minlength=NBLK * NG)
    starts = np.concatenate([[0], np.cumsum(cnt)[:-1]])
    within = np.arange(E, dtype=np.int64) - np.repeat(starts, cnt)

    CH_SB = B * sum(MG)                 # chunks per super-block (133)
    SLOTS_SB = CH_SB * P                # 17024
    SLOTS_CORE = NSB * SLOTS_SB         # 238336
    CH_CORE = NSB * CH_SB
    W_CORE = SLOTS_CORE // 16
    cumMgB = np.concatenate([[0], np.cumsum([B * m for m in MG])])  # chunks
    # slot base for (block b, group g), relative to its core
    b_all = np.arange(NBLK, dtype=np.int64)
    b_in_core = b_all % bpc
    sb_of = b_in_core // B
    bl_of = b_in_core % B
    base_bg = np.empty((NBLK, NG), np.int64)
    for g in range(NG):
        base_bg[:, g] = (sb_of * SLOTS_SB + cumMgB[g] * P
                         + bl_of * MG[g] * P)
    slot_core = base_bg.reshape(-1)[key] + within
    core_of_edge = (eb_s // bpc).astype(np.int64)

    idx16 = np.zeros((ncores, SLOTS_CORE), np.int16)
    locs = np.full((ncores, SLOTS_CORE), 255.0, np.float32)
    flat = core_of_edge * SLOTS_CORE + slot_core
    idx16.reshape(-1)[flat] = (row_s - eg_s * TBL).astype(np.int16)
    locs.reshape(-1)[flat] = el_s

    # idx j -> [j%16, j//16], replicated to 128 partitions
    idx_wrap = idx16.reshape(ncores, W_CORE, 16).transpose(0, 2, 1)
    idx_rep = np.ascontiguousarray(np.tile(idx_wrap, (1, 8, 1)))
    # locs chunk-major: slot = c*128 + p -> locs_T[p, c]
    locs_T = np.ascontiguousarray(
        locs.reshape(ncores, CH_CORE, P).transpose(0, 2, 1)).astype(dt_np)

    # gather tables (node-id order, split by range)
    x_pad = np.zeros((NPAD, P), np.float32)
    x_pad[:N] = x
    xg = np.ascontiguousarray(x_pad.astype(dt_np))
    xg_tables = [xg[0:TBL], xg[TBL:2 * TBL], xg[2 * TBL:3 * TBL],
                 xg[3 * TBL:NPAD]]

    # inverse in-degree per (core, loc, block)
    invdeg = (1.0 / np.maximum(deg, 1.0)).astype(np.float32)
    invdeg_slot = invdeg[node_of_slot]
    invdeg_T = np.ascontiguousarray(
        invdeg_slot.reshape(ncores, bpc, P).transpose(0, 2, 1))

    # per-core transposed x in (block, loc) order
    xt_all = x_pad[node_of_slot].astype(dt_np)
    xt_cores = np.ascontiguousarray(
        xt_all.reshape(ncores, bpc * P, P).transpose(0, 2, 1))

    waggT = np.ascontiguousarray(W_agg.T).astype(dt_np)
    wlinT = np.ascontiguousarray(W_lin.T).astype(dt_np)
    iota = np.tile(np.arange(P, dtype=np.float64), (P, 1)).astype(dt_np)
    ident = np.eye(P, dtype=np.float64).astype(dt_np)

    in_maps = []
    for k in range(ncores):
        m = {
            "idxs": idx_rep[k],
            "locs": locs_T[k],
            "invdeg": invdeg_T[k],
            "xt": xt_cores[k],
            "wagg": waggT,
            "wlin": wlinT,
            "iota": iota,
            "ident": ident,
        }
        for g in range(NG):
            m[f"xg{g}"] = xg_tables[g]
        in_maps.append(m)
    return in_maps, node_of_slot


# ---------------------------------------------------------------- device program

def _build_nc(bpc, dt_np, debug=False):
    import concourse.bass as bass  # noqa: F401
    import concourse.bacc as bacc
    import concourse.mybir as mybir
    import concourse.tile as tile

    dt = mybir.dt.from_np(np.dtype(dt_np))
    f32 = mybir.dt.float32
    NB = bpc
    NCN = NB * P
    NSB = bpc // B
    NG = 4
    CH_SB = B * sum(MG)
    SLOTS_SB = CH_SB * P
    CH_CORE = NSB * CH_SB
    W_CORE = NSB * SLOTS_SB // 16
    cumMgB = [0]
    for m in MG:
        cumMgB.append(cumMgB[-1] + B * m)
    tbl_rows = [TBL, TBL, TBL, NCN * 8 - 3 * TBL]

    nc = bacc.Bacc("TRN2", target_bir_lowering=False, debug=debug,
                   num_swdge_queues=4)

    xg_d = [nc.dram_tensor(f"xg{g}", [tbl_rows[g], P], dt,
                           kind="ExternalInput") for g in range(NG)]
    idxs_d = nc.dram_tensor("idxs", [P, W_CORE], mybir.dt.int16,
                            kind="ExternalInput")
    locs_d = nc.dram_tensor("locs", [P, CH_CORE], dt, kind="ExternalInput")
    invdeg_d = nc.dram_tensor("invdeg", [P, NB], f32, kind="ExternalInput")
    xt_d = nc.dram_tensor("xt", [P, NCN], dt, kind="ExternalInput")
    wagg_d = nc.dram_tensor("wagg", [P, P], dt, kind="ExternalInput")
    wlin_d = nc.dram_tensor("wlin", [P, P], dt, kind="ExternalInput")
    iota_d = nc.dram_tensor("iota", [P, P], dt, kind="ExternalInput")
    ident_d = nc.dram_tensor("ident", [P, P], dt, kind="ExternalInput")
    out_d = nc.dram_tensor("out", [NCN, P], f32, kind="ExternalOutput")

    AF = mybir.ActivationFunctionType
    OP = mybir.AluOpType

    with tile.TileContext(nc) as tc:
        with tc.tile_pool(name="const", bufs=1) as cp, \
             tc.tile_pool(name="msg", bufs=2) as mp, \
             tc.tile_pool(name="spool", bufs=2) as spool, \
             tc.tile_pool(name="xpool", bufs=2) as xp, \
             tc.tile_pool(name="blk", bufs=2) as bp, \
             tc.tile_pool(name="psum", bufs=2, space="PSUM") as pp:

            idxs_t = cp.tile([P, W_CORE], mybir.dt.int16)
            nc.sync.dma_start(out=idxs_t[:], in_=idxs_d[:])
            locs_t = cp.tile([P, CH_CORE], dt)
            nc.sync.dma_start(out=locs_t[:], in_=locs_d[:])
            invdeg_t = cp.tile([P, NB], f32)
            nc.sync.dma_start(out=invdeg_t[:], in_=invdeg_d[:])
            iota_t = cp.tile([P, P], dt)
            nc.sync.dma_start(out=iota_t[:], in_=iota_d[:])
            ident_t = cp.tile([P, P], dt)
            nc.sync.dma_start(out=ident_t[:], in_=ident_d[:])
            wagg_t = cp.tile([P, P], dt)
            nc.sync.dma_start(out=wagg_t[:], in_=wagg_d[:])
            wlin_t = cp.tile([P, P], dt)
            nc.sync.dma_start(out=wlin_t[:], in_=wlin_d[:])

            GCH = 4  # chunks per dma_gather (512 idxs = 33 descs/engine:
                     # two gathers fit in a queue's ring (cap ~72) so descgen
                     # pipelines with the previous gather's transfer)
            qq = 0
            for sb in range(NSB):
                # 1. gather the super-block's source rows: sub-gathers of
                # GCH chunks per range table, spread over the 4 SWDGE queues
                # (each queue = its own Q7 cpu pair + SDMA ring)
                msgs = []
                for g in range(NG):
                    nch = B * MG[g]
                    mg_t = mp.tile([P, nch, P], dt, tag=f"m{g}")
                    off = sb * SLOTS_SB + cumMgB[g] * P
                    for c0 in range(0, nch, GCH):
                        n_sub = min(GCH, nch - c0) * P
                        o = off + c0 * P
                        nc.gpsimd.dma_gather(
                            mg_t[:, c0:c0 + n_sub // P, :], xg_d[g][:],
                            idxs_t[:, o // 16:(o + n_sub) // 16],
                            n_sub, n_sub, P, queue_num=qq % 4)
                        qq += 1
                    msgs.append(mg_t)

                # 2. one-hot S[e, c_chunk, :] = (loc == iota)
                S_t = spool.tile([P, CH_SB, P], dt, tag="S")
                c0 = sb * CH_SB
                nc.vector.tensor_tensor(
                    out=S_t[:],
                    in0=locs_t[:, c0:c0 + CH_SB].to_broadcast([P, CH_SB, P]),
                    in1=iota_t[:, None, :].to_broadcast([P, CH_SB, P]),
                    op=OP.is_equal)

                xt_t = xp.tile([P, B * P], dt, tag="xt")
                nc.sync.dma_start(
                    out=xt_t[:], in_=xt_d[:, sb * B * P:(sb + 1) * B * P])

                for bl in range(B):
                    b = sb * B + bl
                    # 3. scatter-accumulate: acc[c, :] += S_c.T @ msg_c
                    acc_p = pp.tile([P, P], f32, tag="acc")
                    mm, total_mm = 0, sum(MG)
                    for g in range(NG):
                        for m in range(MG[g]):
                            c_local = cumMgB[g] + bl * MG[g] + m
                            nc.tensor.matmul(
                                out=acc_p[:], lhsT=S_t[:, c_local, :],
                                rhs=msgs[g][:, bl * MG[g] + m, :],
                                start=(mm == 0), stop=(mm == total_mm - 1))
                            mm += 1

                    # 4. agg = summed * invdeg
                    agg_t = bp.tile([P, P], dt, tag="agg")
                    nc.vector.tensor_scalar(
                        out=agg_t[:], in0=acc_p[:],
                        scalar1=invdeg_t[:, b:b + 1], scalar2=None,
                        op0=OP.mult)

                    # 5. transpose agg; out = agg @ W_agg.T + x @ W_lin.T
                    aggT_p = pp.tile([P, P], dt, tag="aggTp")
                    nc.tensor.transpose(out=aggT_p[:], in_=agg_t[:],
                                        identity=ident_t[:])
                    aggT_t = bp.tile([P, P], dt, tag="aggT")
                    nc.vector.tensor_copy(out=aggT_t[:], in_=aggT_p[:])
                    out_p = pp.tile([P, P], f32, tag="out")
                    nc.tensor.matmul(out=out_p[:], lhsT=aggT_t[:],
                                     rhs=wagg_t[:], start=True, stop=False)
                    nc.tensor.matmul(out=out_p[:],
                                     lhsT=xt_t[:, bl * P:(bl + 1) * P],
                                     rhs=wlin_t[:], start=False, stop=True)

                    # 6. L2 normalize rows
                    sq_t = bp.tile([P, P], f32, tag="sq")
                    ss_t = bp.tile([P, 1], f32, tag="ss")
                    nc.scalar.activation(out=sq_t[:], in_=out_p[:],
                                         func=AF.Square, accum_out=ss_t[:])
                    nrm_t = bp.tile([P, 1], f32, tag="nrm")
                    nc.scalar.sqrt(out=nrm_t[:], in_=ss_t[:])
                    nrmc_t = bp.tile([P, 1], f32, tag="nrmc")
                    nc.vector.tensor_scalar_max(nrmc_t[:], nrm_t[:], 1e-12)
                    inv_t = bp.tile([P, 1], f32, tag="inv")
                    nc.vector.reciprocal(out=inv_t[:], in_=nrmc_t[:])
                    outs_t = bp.tile([P, P], f32, tag="outs")
                    nc.vector.tensor_scalar(
                        out=outs_t[:], in0=out_p[:],
                        scalar1=inv_t[:, :1], scalar2=None, op0=OP.mult)
                    nc.sync.dma_start(out=out_d[b * P:(b + 1) * P, :],
                                      in_=outs_t[:])

    return nc


# ---------------------------------------------------------------- entry point

def _run(x, W_lin, W_agg, edge_index, ncores, bpc, dt_np, trace=False):
    from concourse import bass_utils

    in_maps, node_of_slot = _host_prep(
        x, W_lin, W_agg, edge_index, ncores, bpc, dt_np)
    nc = _build_nc(bpc, dt_np)
    nc.compile()
    res = bass_utils.run_bass_kernel_spmd(
        nc, in_maps, core_ids=list(range(ncores)), trace=trace)
    outs = np.concatenate([r["out"] for r in res.results], axis=0)
    N = x.shape[0]
    out_pad = np.empty((len(node_of_slot), P), np.float32)
    out_pad[node_of_slot] = outs
    return out_pad[:N], res


def kernel(x, W_lin, W_agg, edge_index):
    import os
    x = np.ascontiguousarray(x, dtype=np.float32)
    W_lin = np.ascontiguousarray(W_lin, dtype=np.float32)
    W_agg = np.ascontiguousarray(W_agg, dtype=np.float32)
    dt_env = os.environ.get("KERNEL_DTYPE", "float16")
    if dt_env == "bfloat16":
        import ml_dtypes
        dt_np = ml_dtypes.bfloat16
    elif dt_env == "float16":
        dt_np = np.float16
    else:
        dt_np = np.float32
    trace = os.environ.get("KERNEL_TRACE", "0") == "1"
    if trace:
        try:
            sys.path.insert(0, os.path.dirname(os.path.abspath(__file__)))
            import ntff_shim  # noqa: F401
        except Exception:
            pass
    out, res = _run(x, W_lin, W_agg, edge_index, ncores=8, bpc=98,
                    dt_np=dt_np, trace=trace)
    if res.exec_time_ns is not None:
        print(f"HW exec time: {res.exec_time_ns} ns")
    return out


# revision 10
# speedup vs baseline: 1.7612x; 1.6457x over previous
"""Trainium2 Bass kernel for CustomSAGEConv (GNN mean-aggregation message passing).

  out = normalize( mean_agg(x[row] -> col) @ W_agg.T + x @ W_lin.T )

Strategy (8 NeuronCores, SPMD single program):
  - Host: partition the 100K nodes into 784 blocks of 128 (8 cores x 98
    blocks), balancing per-(block, source-range-group) in-edge counts under
    exact targets t=(704,704,704,64) slots per block for the 4 source ranges
    (dma_gather uses int16 indices -> x is split into 4 tables of <=32768
    rows). Edges are routed to the core/block owning their dest (col).
    Slot layout per core: [super-block of B=7 blocks][group][block][t_g
    slots]; pad slots carry code 4096 (never matches) and index 0.
  - Device, per super-block:
      1. dma_gather instructions (<=1024 idx each: SWDGE ring holds ~72
         descs/engine) fetch the source rows; gathered row j lands at
         msg[j%128, j//128, :]. 4 SWDGE queues give 4 descriptor rings.
      2. one-hot S built per (group, block j) on DVE from codes
         (128*b_local + loc) vs iota+128j; boundary chunks (t_g not a
         multiple of 128) get one S column per overlapping block.
      3. per block: ~19 matmuls accumulate PSUM[c, :] += S_c.T @ msg_c.
      4. agg = summed * invdeg; PE-transpose; out = agg@W_agg.T + x@W_lin.T;
         row L2-normalize; DMA out.
  - Host: inverse-permute rows back to original node order.
"""

import sys

sys.path.insert(0, "/opt/trn_rl_repo")

import numpy as np

P = 128
TBL = 32768
NG = 4
T_G = (704, 704, 704, 64)   # slot target per (block, group)
B = 7                       # blocks per super-block
GCH = 8                     # chunks per dma_gather (1024 idx = 65 descs, max
                            # that fits the ~72-desc SWDGE ring)
PADCODE = 4096.0

# derived layout (per super-block)
RUN_G = [((B * t + P - 1) // P) * P for t in T_G]      # padded run slots
NCH_G = [r // P for r in RUN_G]                        # chunks per (SB, g)
CUM_RUN = [0]
for r in RUN_G:
    CUM_RUN.append(CUM_RUN[-1] + r)
SLOTS_SB = CUM_RUN[-1]
CUM_CH = [0]
for n in NCH_G:
    CUM_CH.append(CUM_CH[-1] + n)
CH_SB = CUM_CH[-1]

# per (g, block j): chunk range [cl, ch] of block j within the group's run
CLCH = []
for g in range(NG):
    t = T_G[g]
    rows = []
    for j in range(B):
        cl = (j * t) // P
        ch = min(((j + 1) * t - 1) // P, NCH_G[g] - 1)
        if j == B - 1:
            ch = NCH_G[g] - 1          # tail pad chunks belong to last block
        rows.append((cl, ch))
    CLCH.append(rows)
# S pair layout: per g, pairs grouped by j (boundary chunks duplicated)
OFF_GJ = []
PAIRBASE_G = [0]
for g in range(NG):
    offs, acc = [], 0
    for j in range(B):
        cl, ch = CLCH[g][j]
        offs.append(acc)
        acc += ch - cl + 1
    OFF_GJ.append(offs)
    PAIRBASE_G.append(PAIRBASE_G[-1] + acc)
PAIRS_SB = PAIRBASE_G[-1]


# ---------------------------------------------------------------- host prep

def _host_prep(x, W_lin, W_agg, edge_index, ncores, bpc, dt_np):
    """Build per-core device inputs."""
    N, D = x.shape
    assert D == P
    NBLK = ncores * bpc
    NPAD = NBLK * P
    assert N <= NPAD
    assert bpc % B == 0
    NSB = bpc // B

    row = np.ascontiguousarray(edge_index[0]).astype(np.int32)
    col = np.ascontiguousarray(edge_index[1]).astype(np.int32)
    E = row.shape[0]
    grp = (row // TBL).astype(np.int32)
    assert grp.max() < NG

    # per-node per-group in-degree
    indeg4 = np.bincount(col.astype(np.int64) * NG + grp,
                         minlength=NPAD * NG).reshape(NPAD, NG)
    deg = indeg4.sum(1)

    # initial node->block: degree-sorted snake round robin
    order = np.argsort(-deg, kind="stable")
    seq = np.arange(NPAD, dtype=np.int64)
    cyc, pos = seq // NBLK, seq % NBLK
    snake = np.where(cyc % 2 == 0, pos, NBLK - 1 - pos).astype(np.int32)
    blk_of = np.empty(NPAD, np.int32)
    blk_of[order] = snake

    n_bg = np.bincount(blk_of[col].astype(np.int64) * NG + grp,
                       minlength=NBLK * NG).reshape(NBLK, NG)
    caps = np.array(T_G, np.int64)
    for it in range(20000):
        over = n_bg - caps[None, :]
        flat = int(np.argmax(over))
        b_hi, g = flat // NG, flat % NG
        if over[b_hi, g] <= 0:
            break
        nodes_hi = np.where(blk_of == b_hi)[0]
        n1 = nodes_hi[np.argmax(indeg4[nodes_hi, g])]
        done = False
        for b_lo in np.argsort(n_bg[:, g])[:48]:
            if b_lo == b_hi:
                continue
            nodes_lo = np.where(blk_of == b_lo)[0]
            cand = nodes_lo[np.argsort(indeg4[nodes_lo, g])[:8]]
            for n2 in cand:
                d = indeg4[n1] - indeg4[n2]
                if d[g] <= 0:
                    continue
                # b_lo must stay legal; b_hi must not grow a NEW violation
                if np.all(n_bg[b_lo] + d <= caps) and \
                   np.all(n_bg[b_hi] - d <= np.maximum(caps, n_bg[b_hi])):
                    blk_of[n1], blk_of[n2] = b_lo, b_hi
                    n_bg[b_hi] -= d
                    n_bg[b_lo] += d
                    done = True
                    break
            if done:
                break
        if not done:
            raise RuntimeError(
                f"balance repair stuck at iter {it}: {n_bg[b_hi]}, cap {caps}")
    assert np.all(n_bg <= caps[None, :]), "balance failed"

    # node -> (block, loc); slot->node map
    o2 = np.argsort(blk_of, kind="stable")
    loc_of = np.empty(NPAD, np.int32)
    loc_of[o2] = (np.arange(NPAD, dtype=np.int64) % P).astype(np.int32)
    node_of_slot = o2

    # ---- edge slot placement -------------------------------------------
    eb = blk_of[col]
    el = loc_of[col]
    eo = np.lexsort((row, grp, eb))
    eb_s, el_s, row_s, eg_s = eb[eo], el[eo], row[eo], grp[eo]

    key = eb_s.astype(np.int64) * NG + eg_s
    cnt = np.bincount(key, minlength=NBLK * NG)
    starts = np.concatenate([[0], np.cumsum(cnt)[:-1]])
    within = np.arange(E, dtype=np.int64) - np.repeat(starts, cnt)

    SLOTS_CORE = NSB * SLOTS_SB
    CH_CORE = NSB * CH_SB
    W_CORE = SLOTS_CORE // 16

    b_all = np.arange(NBLK, dtype=np.int64)
    b_in_core = b_all % bpc
    sb_of = b_in_core // B
    bl_of = b_in_core % B
    base_bg = np.empty((NBLK, NG), np.int64)
    for g in range(NG):
        base_bg[:, g] = sb_of * SLOTS_SB + CUM_RUN[g] + bl_of * T_G[g]
    slot_core = base_bg.reshape(-1)[key] + within
    core_of_edge = (eb_s // bpc).astype(np.int64)

    idx16 = np.zeros((ncores, SLOTS_CORE), np.int16)
    codes = np.full((ncores, SLOTS_CORE), PADCODE, np.float32)
    flat = core_of_edge * SLOTS_CORE + slot_core
    idx16.reshape(-1)[flat] = (row_s - eg_s * TBL).astype(np.int16)
    codes.reshape(-1)[flat] = el_s + P * bl_of[eb_s]

    idx_wrap = idx16.reshape(ncores, W_CORE, 16).transpose(0, 2, 1)
    idx_rep = np.ascontiguousarray(np.tile(idx_wrap, (1, 8, 1)))
    locs_T = np.ascontiguousarray(
        codes.reshape(ncores, CH_CORE, P).transpose(0, 2, 1)).astype(dt_np)

    # gather tables (node-id order, split by range)
    x_pad = np.zeros((NPAD, P), np.float32)
    x_pad[:N] = x
    xg = np.ascontiguousarray(x_pad.astype(dt_np))
    xg_tables = [xg[0:TBL], xg[TBL:2 * TBL], xg[2 * TBL:3 * TBL],
                 xg[3 * TBL:NPAD]]

    invdeg = (1.0 / np.maximum(deg, 1.0)).astype(np.float32)
    invdeg_slot = invdeg[node_of_slot]
    invdeg_T = np.ascontiguousarray(
        invdeg_slot.reshape(ncores, bpc, P).transpose(0, 2, 1))

    xt_all = x_pad[node_of_slot].astype(dt_np)
    xt_cores = np.ascontiguousarray(
        xt_all.reshape(ncores, bpc * P, P).transpose(0, 2, 1))

    waggT = np.ascontiguousarray(W_agg.T).astype(dt_np)
    wlinT = np.ascontiguousarray(W_lin.T).astype(dt_np)
    # iota7[p, j*128 + c] = j*128 + c  (compare target for block j)
    iota7 = np.tile(np.arange(B * P, dtype=np.float64), (P, 1)).astype(dt_np)
    ident = np.eye(P, dtype=np.float64).astype(dt_np)

    in_maps = []
    for k in range(ncores):
        m = {
            "idxs": idx_rep[k],
            "locs": locs_T[k],
            "invdeg": invdeg_T[k],
            "xt": xt_cores[k],
            "wagg": waggT,
            "wlin": wlinT,
            "iota7": iota7,
            "ident": ident,
        }
        for g in range(NG):
            m[f"xg{g}"] = xg_tables[g]
        in_maps.append(m)
    return in_maps, node_of_slot


# ---------------------------------------------------------------- device program

def _build_nc(bpc, dt_np, debug=False):
    import concourse.bass as bass  # noqa: F401
    import concourse.bacc as bacc
    import concourse.mybir as mybir
    import concourse.tile as tile

    dt = mybir.dt.from_np(np.dtype(dt_np))
    f32 = mybir.dt.float32
    NB = bpc
    NCN = NB * P
    NSB = bpc // B
    CH_CORE = NSB * CH_SB
    W_CORE = NSB * SLOTS_SB // 16
    tbl_rows = [TBL, TBL, TBL, NCN * 8 - 3 * TBL]

    nc = bacc.Bacc("TRN2", target_bir_lowering=False, debug=debug,
                   num_swdge_queues=4)

    xg_d = [nc.dram_tensor(f"xg{g}", [tbl_rows[g], P], dt,
                           kind="ExternalInput") for g in range(NG)]
    idxs_d = nc.dram_tensor("idxs", [P, W_CORE], mybir.dt.int16,
                            kind="ExternalInput")
    locs_d = nc.dram_tensor("locs", [P, CH_CORE], dt, kind="ExternalInput")
    invdeg_d = nc.dram_tensor("invdeg", [P, NB], f32, kind="ExternalInput")
    xt_d = nc.dram_tensor("xt", [P, NCN], dt, kind="ExternalInput")
    wagg_d = nc.dram_tensor("wagg", [P, P], dt, kind="ExternalInput")
    wlin_d = nc.dram_tensor("wlin", [P, P], dt, kind="ExternalInput")
    iota7_d = nc.dram_tensor("iota7", [P, B * P], dt, kind="ExternalInput")
    ident_d = nc.dram_tensor("ident", [P, P], dt, kind="ExternalInput")
    out_d = nc.dram_tensor("out", [NCN, P], f32, kind="ExternalOutput")

    AF = mybir.ActivationFunctionType
    OP = mybir.AluOpType

    with tile.TileContext(nc) as tc:
        with tc.tile_pool(name="const", bufs=1) as cp, \
             tc.tile_pool(name="msg", bufs=2) as mp, \
             tc.tile_pool(name="spool", bufs=2) as spool, \
             tc.tile_pool(name="xpool", bufs=2) as xp, \
             tc.tile_pool(name="blk", bufs=2) as bp, \
             tc.tile_pool(name="psum", bufs=2, space="PSUM") as pp:

            idxs_t = cp.tile([P, W_CORE], mybir.dt.int16)
            nc.sync.dma_start(out=idxs_t[:], in_=idxs_d[:])
            locs_t = cp.tile([P, CH_CORE], dt)
            nc.sync.dma_start(out=locs_t[:], in_=locs_d[:])
            invdeg_t = cp.tile([P, NB], f32)
            nc.sync.dma_start(out=invdeg_t[:], in_=invdeg_d[:])
            iota7_t = cp.tile([P, B * P], dt)
            nc.sync.dma_start(out=iota7_t[:], in_=iota7_d[:])
            ident_t = cp.tile([P, P], dt)
            nc.sync.dma_start(out=ident_t[:], in_=ident_d[:])
            wagg_t = cp.tile([P, P], dt)
            nc.sync.dma_start(out=wagg_t[:], in_=wagg_d[:])
            wlin_t = cp.tile([P, P], dt)
            nc.sync.dma_start(out=wlin_t[:], in_=wlin_d[:])

            qq = 0
            for sb in range(NSB):
                # 1. gather the super-block's source rows
                msgs = []
                for g in range(NG):
                    nch = NCH_G[g]
                    mg_t = mp.tile([P, nch, P], dt, tag=f"m{g}")
                    off = sb * SLOTS_SB + CUM_RUN[g]
                    for c0 in range(0, nch, GCH):
                        n_sub = min(GCH, nch - c0) * P
                        o = off + c0 * P
                        nc.gpsimd.dma_gather(
                            mg_t[:, c0:c0 + n_sub // P, :], xg_d[g][:],
                            idxs_t[:, o // 16:(o + n_sub) // 16],
                            n_sub, n_sub, P, queue_num=qq % 4)
                        qq += 1
                    msgs.append(mg_t)

                # 2. one-hot S per (group, block j): code == iota + 128*j
                S_t = spool.tile([P, PAIRS_SB, P], dt, tag="S")
                c0_sb = sb * CH_SB
                for g in range(NG):
                    for j in range(B):
                        cl, ch = CLCH[g][j]
                        n = ch - cl + 1
                        pb = PAIRBASE_G[g] + OFF_GJ[g][j]
                        lo = c0_sb + CUM_CH[g] + cl
                        nc.vector.tensor_tensor(
                            out=S_t[:, pb:pb + n, :],
                            in0=locs_t[:, lo:lo + n].to_broadcast([P, n, P]),
                            in1=iota7_t[:, None, j * P:(j + 1) * P]
                                .to_broadcast([P, n, P]),
                            op=OP.is_equal)

                xt_t = xp.tile([P, B * P], dt, tag="xt")
                nc.sync.dma_start(
                    out=xt_t[:], in_=xt_d[:, sb * B * P:(sb + 1) * B * P])

                for j in range(B):
                    b = sb * B + j
                    # 3. scatter-accumulate: acc[c, :] += S_c.T @ msg_c
                    acc_p = pp.tile([P, P], f32, tag="acc")
                    total_mm = sum(CLCH[g][j][1] - CLCH[g][j][0] + 1
                                   for g in range(NG))
                    mm = 0
                    for g in range(NG):
                        cl, ch = CLCH[g][j]
                        pb = PAIRBASE_G[g] + OFF_GJ[g][j]
                        for c in range(cl, ch + 1):
                            nc.tensor.matmul(
                                out=acc_p[:],
                                lhsT=S_t[:, pb + (c - cl), :],
                                rhs=msgs[g][:, c, :],
                                start=(mm == 0), stop=(mm == total_mm - 1))
                            mm += 1

                    # 4. agg = summed * invdeg
                    agg_t = bp.tile([P, P], dt, tag="agg")
                    nc.vector.tensor_scalar(
                        out=agg_t[:], in0=acc_p[:],
                        scalar1=invdeg_t[:, b:b + 1], scalar2=None,
                        op0=OP.mult)

                    # 5. transpose agg; out = agg @ W_agg.T + x @ W_lin.T
                    aggT_p = pp.tile([P, P], dt, tag="aggTp")
                    nc.tensor.transpose(out=aggT_p[:], in_=agg_t[:],
                                        identity=ident_t[:])
                    aggT_t = bp.tile([P, P], dt, tag="aggT")
                    nc.vector.tensor_copy(out=aggT_t[:], in_=aggT_p[:])
                    out_p = pp.tile([P, P], f32, tag="out")
                    nc.tensor.matmul(out=out_p[:], lhsT=aggT_t[:],
                                     rhs=wagg_t[:], start=True, stop=False)
                    nc.tensor.matmul(out=out_p[:],
                                     lhsT=xt_t[:, j * P:(j + 1) * P],
                                     rhs=wlin_t[:], start=False, stop=True)

                    # 6. L2 normalize rows
                    sq_t = bp.tile([P, P], f32, tag="sq")
                    ss_t = bp.tile([P, 1], f32, tag="ss")
                    nc.scalar.activation(out=sq_t[:], in_=out_p[:],
                                         func=AF.Square, accum_out=ss_t[:])
                    nrm_t = bp.tile([P, 1], f32, tag="nrm")
                    nc.scalar.sqrt(out=nrm_t[:], in_=ss_t[:])
                    nrmc_t = bp.tile([P, 1], f32, tag="nrmc")
                    nc.vector.tensor_scalar_max(nrmc_t[:], nrm_t[:], 1e-12)
                    inv_t = bp.tile([P, 1], f32, tag="inv")
                    nc.vector.reciprocal(out=inv_t[:], in_=nrmc_t[:])
                    outs_t = bp.tile([P, P], f32, tag="outs")
                    nc.vector.tensor_scalar(
                        out=outs_t[:], in0=out_p[:],
                        scalar1=inv_t[:, :1], scalar2=None, op0=OP.mult)
                    nc.sync.dma_start(out=out_d[b * P:(b + 1) * P, :],
                                      in_=outs_t[:])

    return nc


# ---------------------------------------------------------------- entry point

def _run(x, W_lin, W_agg, edge_index, ncores, bpc, dt_np, trace=False):
    from concourse import bass_utils

    in_maps, node_of_slot = _host_prep(
        x, W_lin, W_agg, edge_index, ncores, bpc, dt_np)
    nc = _build_nc(bpc, dt_np)
    nc.compile()
    res = bass_utils.run_bass_kernel_spmd(
        nc, in_maps, core_ids=list(range(ncores)), trace=trace)
    outs = np.concatenate([r["out"] for r in res.results], axis=0)
    N = x.shape[0]
    out_pad = np.empty((len(node_of_slot), P), np.float32)
    out_pad[node_of_slot] = outs
    return out_pad[:N], res


def kernel(x, W_lin, W_agg, edge_index):
    import os
    x = np.ascontiguousarray(x, dtype=np.float32)
    W_lin = np.ascontiguousarray(W_lin, dtype=np.float32)
    W_agg = np.ascontiguousarray(W_agg, dtype=np.float32)
    dt_env = os.environ.get("KERNEL_DTYPE", "float16")
    if dt_env == "bfloat16":
        import ml_dtypes
        dt_np = ml_dtypes.bfloat16
    elif dt_env == "float16":
        dt_np = np.float16
    else:
        dt_np = np.float32
    trace = os.environ.get("KERNEL_TRACE", "0") == "1"
    if trace:
        try:
            sys.path.insert(0, os.path.dirname(os.path.abspath(__file__)))
            import ntff_shim  # noqa: F401
        except Exception:
            pass
    out, res = _run(x, W_lin, W_agg, edge_index, ncores=8, bpc=98,
                    dt_np=dt_np, trace=trace)
    if res.exec_time_ns is not None:
        print(f"HW exec time: {res.exec_time_ns} ns")
    return out
